# revision 24
# baseline (speedup 1.0000x reference)
"""GQA kernel for Trainium2, 8 NeuronCores.

Sharding: core c = (b, g) with b = c // 4 (batch), g = c % 4 (KV group).
Each core computes, for its batch b and group g (4 query heads, 1 KV head):
  qT[d, t] for the 4 heads, kT[d, t], v[t, d] projections (contraction over EMB,
  inputs pre-packed on host so EMB lands on SBUF partitions),
  causal flash-style attention in [k-part, q-free] score layout,
  and the partial output projection  partial_g = (attn out) @ Wp[:, g cols].T.
Host gathers: y[b] = sum_g partial[b, g] + bp.

All matmuls run in bf16 (fp32 PSUM accumulation); host pre-casts inputs.
Inputs are host-packed to the exact SBUF layout [128, free] so each tensor
loads with one contiguous DMA (dma_start issue cost dominates chunked loads).
Causal structure: scores for the diagonal 128-row k-blocks are trimmed to the
q-columns that can attend; only the first 128-col band of each diagonal block
needs an elementwise triangular mask (applied on GPSIMD, off the DVE).

Score tiles are PAIRED into [128, 1024] two-bank PSUM tiles so one ACT
activation exponentiates two k-blocks at once — the per-instruction 352-cycle
ACT overhead otherwise makes exp the late-q-pass bottleneck.

Output partials are written bf16 (halves output DMA bytes + drain tail), with
one staged [128, 2048] DMA per 128-row tile instead of four 512-col DMAs
(HWDGE trigger instructions cost ~0.6us each on the issuing engine).
"""

import numpy as np
import ml_dtypes

T = 2048
EMB = 2048
HD = 128
GS = 4          # query heads per core (per KV group)
NE = EMB // 128 # 16 contraction chunks
NT = T // 128   # 16 row tiles
NQP = T // 512  # 4 q passes of 512
SCALE = float(HD) ** -0.5

_BF16 = ml_dtypes.bfloat16
_PROGRAM = None


def _build_program():
    import concourse.bass as bass
    import concourse.tile as tile
    from concourse import bacc, mybir
    from concourse.masks import make_identity

    f32 = mybir.dt.float32
    bf16 = mybir.dt.bfloat16

    nc = bacc.Bacc("TRN2", target_bir_lowering=False, debug=False)

    # all inputs host-packed to [128 partitions, free] SBUF layout
    xT_d = nc.dram_tensor("xTp", [128, NE * T], bf16, kind="ExternalInput")
    wq_d = nc.dram_tensor("wqp", [128, NE * GS * HD], bf16, kind="ExternalInput")
    wk_d = nc.dram_tensor("wkp", [128, NE * HD], bf16, kind="ExternalInput")
    wv_d = nc.dram_tensor("wvp", [128, NE * HD], bf16, kind="ExternalInput")
    wp_d = nc.dram_tensor("wpp", [128, GS * EMB], bf16, kind="ExternalInput")
    out_d = nc.dram_tensor("partial", [T, EMB], bf16, kind="ExternalOutput").rearrange(
        "(n p) m -> n p m", p=128
    )

    with tile.TileContext(nc) as tc:
        with (
            tc.tile_pool(name="big", bufs=1) as big,
            tc.tile_pool(name="pt", bufs=20) as ptp,
            tc.tile_pool(name="onorm", bufs=16) as onp,
            tc.tile_pool(name="ostage", bufs=2) as osp,
            tc.tile_pool(name="small", bufs=6) as smp,
            tc.tile_pool(name="mm", bufs=2, space="PSUM") as pmm,
            tc.tile_pool(name="oext", bufs=2, space="PSUM") as pox,
            tc.tile_pool(name="tr", bufs=2, space="PSUM") as ptr,
        ):
            xT_sb = big.tile([128, NE * T], bf16)
            wq_sb = big.tile([128, NE * GS * HD], bf16)
            wk_sb = big.tile([128, NE * HD], bf16)
            wv_sb = big.tile([128, NE * HD], bf16)
            wp_sb = big.tile([128, GS * EMB], bf16)
            qT_sb = big.tile([128, GS * T], bf16)
            kT_sb = big.tile([128, T], bf16)
            vT_sb = big.tile([128, T], bf16)
            vext_sb = big.tile([128, NT * (HD + 1)], bf16)
            ohT_sb = big.tile([128, GS * T], bf16)
            ident = big.tile([128, 128], bf16)
            mask = big.tile([128, 128], bf16)

            # input DMAs: contiguous block loads; xT quarters gate the
            # projection chains so they go first on the sync queue
            nc.scalar.dma_start(out=wk_sb, in_=wk_d[:, :])
            nc.scalar.dma_start(out=wv_sb, in_=wv_d[:, :])
            # wq/wp ride the same (sync) queue BEHIND all xT chunks: they are
            # not needed until the qT/output projections, and issuing them
            # early would steal HBM bandwidth from the arrival-critical xT.
            # The first chunks are split finer so the PE's first real matmuls
            # start as early as possible — they double as the HAM clock
            # warmup (no garbage warmup matmuls needed).
            splits = {0: 4, 1: 2}
            for c in range(NE):
                n = splits.get(c, 1)
                w = T // n
                for i in range(n):
                    nc.sync.dma_start(
                        out=xT_sb[:, c * T + i * w : c * T + (i + 1) * w],
                        in_=xT_d[:, c * T + i * w : c * T + (i + 1) * w],
                    )
            hw = NE * GS * HD // 2
            for q in range(2):
                nc.sync.dma_start(
                    out=wq_sb[:, q * hw : (q + 1) * hw],
                    in_=wq_d[:, q * hw : (q + 1) * hw],
                )
            nc.sync.dma_start(out=wp_sb, in_=wp_d[:, :])

            # PE clock warmup: 8 cold matmuls (~3.4us @ 1.2 GHz) on garbage
            # SBUF data, filling the otherwise-idle DMA-latency window before
            # the first x quarter-chunk lands, so HAM trips to 2.4 GHz right
            # as the first real matmuls start. Garbage weights (not identity)
            # so nothing gates on the mask/identity memsets.
            wps = pmm.tile([128, 1024], f32, tag="mm", name="wps")
            for w in range(8):
                nc.tensor.matmul(
                    wps[:, 0:512],
                    lhsT=ohT_sb[:, 1024:1152],
                    rhs=ohT_sb[:, 0:512],
                    start=True,
                    stop=True,
                )

            # constants: identity for PE transpose; triangular mask for the
            # first 128-col band of diagonal blocks (keep iff q_local >= k_local)
            make_identity(nc, ident)
            nc.gpsimd.memset(mask, 1.0)
            nc.gpsimd.affine_select(
                out=mask,
                in_=mask,
                compare_op=mybir.AluOpType.is_ge,
                fill=0.0,
                base=0,
                pattern=[[1, 128]],
                channel_multiplier=-1,
            )
            nc.vector.memset(vext_sb, 1.0)

            # kT + vT projections interleaved, chunk-outer so PE consumes each
            # xT chunk as it arrives. kT uses the 4 half-slots of the two "mm"
            # psum tiles; vT borrows the attention pools' slots (oext x2 +
            # tr x2) so both run during the DMA-arrival window.
            kss2 = [pmm.tile([128, 1024], f32, tag="mm", name=f"kss{i}") for i in range(2)]
            kss = [kss2[i // 2][:, (i % 2) * 512 : (i % 2 + 1) * 512] for i in range(4)]
            vss = [
                pox.tile([128, 512], f32, tag="oext", name="vss0"),
                pox.tile([128, 512], f32, tag="oext", name="vss1"),
                ptr.tile([128, 512], f32, tag="tr", name="vss2"),
                ptr.tile([128, 512], f32, tag="tr", name="vss3"),
            ]
            for c in range(NE):
                for tp in range(4):
                    nc.tensor.matmul(
                        kss[tp],
                        lhsT=wk_sb[:, c * HD : (c + 1) * HD],
                        rhs=xT_sb[:, c * T + tp * 512 : c * T + (tp + 1) * 512],
                        start=(c == 0),
                        stop=(c == NE - 1),
                    )
                for tp in range(4):
                    nc.tensor.matmul(
                        vss[tp],
                        lhsT=wv_sb[:, c * HD : (c + 1) * HD],
                        rhs=xT_sb[:, c * T + tp * 512 : c * T + (tp + 1) * 512],
                        start=(c == 0),
                        stop=(c == NE - 1),
                    )
            # wide PSUM->SBUF drains, split scalar/vector so the two pmm
            # tiles free in parallel (a serial 4x720ns scalar drain stalls
            # the next consumer of the pool ~1.5us)
            nc.scalar.copy(kT_sb[:, 0:1024], kss2[0])
            nc.vector.tensor_copy(kT_sb[:, 1024:2048], kss2[1])
            for tp in range(4):
                if tp % 2 == 0:
                    nc.scalar.copy(vT_sb[:, tp * 512 : (tp + 1) * 512], vss[tp])
                else:
                    nc.vector.tensor_copy(vT_sb[:, tp * 512 : (tp + 1) * 512], vss[tp])
            for tt in range(NT):
                tv = ptr.tile([128, 128], bf16, tag="tr")
                nc.tensor.transpose(tv, vT_sb[:, tt * 128 : (tt + 1) * 128], ident)
                nc.vector.tensor_copy(
                    vext_sb[:, tt * (HD + 1) : tt * (HD + 1) + HD], tv
                )

            # qT projection per head: 4 psum chains as 2 tiles x 2 halves.
            # Tile0's chains run (and drain) fully before tile1's start, so
            # each tile's wide PSUM->SBUF copy hides under the other tile's
            # 32 matmuls instead of stalling the next pool user.
            for s in range(GS):
                pss2 = [pmm.tile([128, 1024], f32, tag="mm", name=f"pss{i}") for i in range(2)]
                pss = [pss2[i // 2][:, (i % 2) * 512 : (i % 2 + 1) * 512] for i in range(4)]
                for half in range(2):
                    for c in range(NE):
                        for tp in (2 * half, 2 * half + 1):
                            nc.tensor.matmul(
                                pss[tp],
                                lhsT=wq_sb[
                                    :, c * GS * HD + s * HD : c * GS * HD + (s + 1) * HD
                                ],
                                rhs=xT_sb[:, c * T + tp * 512 : c * T + (tp + 1) * 512],
                                start=(c == 0),
                                stop=(c == NE - 1),
                            )
                    if half == 0:
                        nc.scalar.copy(qT_sb[:, s * T : s * T + 1024], pss2[0])
                    else:
                        nc.vector.tensor_copy(
                            qT_sb[:, s * T + 1024 : s * T + 2048], pss2[1]
                        )

            # attention + output projection, software-pipelined: scores for
            # iteration i+1 are emitted before AV of iteration i so the PE
            # stream never waits for ACT's exp backlog at AV chain heads
            def emit_scores(qp, s):
                # pairs of k-blocks share one [128, 1024] two-bank PSUM tile;
                # one ACT exp covers both (352-cycle ACT overhead amortized)
                nblk = 4 * qp + 4
                pts = [None] * nblk
                for p in range(nblk // 2):
                    ps2 = pmm.tile([128, 1024], f32, tag="mm", name="ps2")
                    info = []
                    for h in range(2):
                        j = 2 * p + h
                        o = j - 4 * qp  # diagonal offset; <0 for full blocks
                        trim = 128 * o if o > 0 else 0
                        w = 512 - trim
                        q0 = s * T + qp * 512 + trim
                        nc.tensor.matmul(
                            ps2[:, h * 512 : h * 512 + w],
                            lhsT=kT_sb[:, j * 128 : (j + 1) * 128],
                            rhs=qT_sb[:, q0 : q0 + w],
                            start=True,
                            stop=True,
                        )
                        info.append((j, o, trim, w))
                    # exp through the end of the second block's valid region
                    ew = 512 + info[1][3]
                    pt2 = ptp.tile([128, 1024], bf16, tag="pt", name="pt2")
                    nc.scalar.activation(
                        pt2[:, 0:ew], ps2[:, 0:ew],
                        mybir.ActivationFunctionType.Exp, scale=SCALE
                    )
                    for h, (j, o, trim, w) in enumerate(info):
                        if o >= 0:
                            # only the first 128-col band straddles the diagonal;
                            # gpsimd keeps this off the loaded DVE
                            nc.gpsimd.tensor_mul(
                                pt2[:, h * 512 : h * 512 + 128],
                                pt2[:, h * 512 : h * 512 + 128],
                                mask,
                            )
                        pts[j] = (pt2, h * 512, trim)
                return pts

            def emit_av(qp, s, pts):
                norms = []
                for u in range(4):
                    jmax = 4 * qp + u
                    oe = pox.tile([128, HD + 1], f32, tag="oext", name="oe")
                    for j in range(jmax + 1):
                        pt2, off, trim = pts[j]
                        c0 = off + u * 128 - trim
                        nc.tensor.matmul(
                            oe,
                            lhsT=pt2[:, c0 : c0 + 128],
                            rhs=vext_sb[:, j * (HD + 1) : (j + 1) * (HD + 1)],
                            start=(j == 0),
                            stop=(j == jmax),
                        )
                    rc = smp.tile([128, 1], f32, tag="rc", name="rc")
                    nc.vector.reciprocal(rc, oe[:, HD : HD + 1])
                    on = onp.tile([128, 128], bf16, tag="on", name="on")
                    nc.vector.tensor_scalar_mul(on, oe[:, 0:HD], rc)
                    norms.append((on, s, qp * 512 + u * 128))
                return norms

            def emit_epilogue_u(qp, u):
                # output projection for one 128-row tile of q-pass qp; stage
                # the full 2048-col bf16 row tile and drain it with ONE dma
                tt = qp * 4 + u
                last = qp == NQP - 1 and u == 3
                ot = osp.tile([128, 2048], bf16, tag="ostage", name="ot")
                for jp in range(4):
                    ps = pox.tile([128, 512], f32, tag="oext", name="ps")
                    for s in range(GS):
                        nc.tensor.matmul(
                            ps,
                            lhsT=ohT_sb[:, s * T + tt * 128 : s * T + (tt + 1) * 128],
                            rhs=wp_sb[:, s * EMB + jp * 512 : s * EMB + (jp + 1) * 512],
                            start=(s == 0),
                            stop=(s == GS - 1),
                        )
                    nc.vector.tensor_copy(ot[:, jp * 512 : (jp + 1) * 512], ps)
                    if last:
                        # final row tile: drain per-512-col slice as each
                        # lands so the very last transfer is small (one
                        # 0.5MB DMA would sit entirely after last compute)
                        eng = nc.sync if jp % 2 == 0 else nc.scalar
                        eng.dma_start(
                            out=out_d[tt, :, jp * 512 : (jp + 1) * 512],
                            in_=ot[:, jp * 512 : (jp + 1) * 512],
                        )
                if not last:
                    eng = nc.sync if u % 2 == 0 else nc.scalar
                    eng.dma_start(out=out_d[tt, :, :], in_=ot)

            last_norms = []
            pending_ep = []

            def emit_transposes(norms):
                for on, s, tq in norms:
                    tps = ptr.tile([128, 128], bf16, tag="tr", name="tps")
                    nc.tensor.transpose(tps, on, ident)
                    nc.vector.tensor_copy(
                        ohT_sb[:, s * T + tq : s * T + tq + 128], tps
                    )

            def advance(pending):
                # AV for the pending iteration, then the (lag-1) transposes of
                # the previous one; at a q-pass boundary flush the transposes
                # and ENQUEUE the pass's 4 output-projection row tiles — one
                # is emitted per advance so the epilogue's bulk PE work is
                # spread across the next pass's iterations, giving ACT's exp
                # stream steady catch-up windows instead of one burst
                nonlocal last_norms
                qp, s, pts = pending
                norms = emit_av(qp, s, pts)
                emit_transposes(last_norms)
                last_norms = norms
                if s == GS - 1:
                    emit_transposes(last_norms)
                    last_norms = []
                    pending_ep.extend((qp, u) for u in range(4))
                if pending_ep:
                    emit_epilogue_u(*pending_ep.pop(0))

            # advance BEFORE emitting the next scores (instead of after) so a
            # pt-pool slot reused by scores(i) was freed by an AV already in
            # the PE queue — lets the pool run 2 iterations deep (20 bufs)
            window = []
            for qp in range(NQP):
                for s in range(GS):
                    if len(window) >= 2:
                        advance(window.pop(0))
                    pts = emit_scores(qp, s)
                    window.append((qp, s, pts))
            for w in window:
                advance(w)
            for qp_u in pending_ep:
                emit_epilogue_u(*qp_u)

    nc.finalize()
    return nc


def _get_program():
    global _PROGRAM
    if _PROGRAM is None:
        _PROGRAM = _build_program()
    return _PROGRAM


def _pack(a, nchunk):
    """[nchunk*128, F] -> [128, nchunk*F] so it lands in SBUF layout with one
    contiguous DMA: out[p, c*F + f] = a[c*128 + p, f]."""
    n, f = a.shape
    assert n == nchunk * 128
    return np.ascontiguousarray(
        a.reshape(nchunk, 128, f).transpose(1, 0, 2).reshape(128, nchunk * f)
    )


def _make_in_maps(x, Wq, Wk, Wv, Wp):
    # convert to numpy up front: slicing a jax array would trace/compile
    # a jax op per slice instead of cheap host-side numpy views
    x, Wq, Wk, Wv, Wp = (np.asarray(a) for a in (x, Wq, Wk, Wv, Wp))
    in_maps = []
    xTs = [_pack(x[b].T.astype(_BF16), NE) for b in range(2)]
    for c in range(8):
        b, g = c // 4, c % 4
        sl = slice(g * GS * HD, (g + 1) * GS * HD)
        kv = slice(g * GS * HD, g * GS * HD + HD)
        in_maps.append(
            {
                "xTp": xTs[b],
                "wqp": _pack(Wq[sl, :].T.astype(_BF16), NE),
                "wkp": _pack(Wk[kv, :].T.astype(_BF16), NE),
                "wvp": _pack(Wv[kv, :].T.astype(_BF16), NE),
                "wpp": _pack(Wp[:, sl].T.astype(_BF16), GS),
            }
        )
    return in_maps


def run(x, Wq, Wk, Wv, Wp, bp, trace=False, **trace_kwargs):
    from concourse.bass_utils import run_bass_kernel_spmd
    from concourse.compiler_utils import temporarily_append_compiler_flags

    nc = _get_program()
    in_maps = _make_in_maps(x, Wq, Wk, Wv, Wp)
    # re-enable the backend's LDWEIGHTS optimization for this kernel's
    # compile: the AV phase issues one fresh 128-col weight load per matmul
    # and is LDWEIGHTS-throughput-bound without it
    with temporarily_append_compiler_flags(
        [
            "--internal-backend-options=--enable-neff-debug-info=true "
            "--dump-on-error --enable-ldw-opt=true "
            "--assign-static-dmas-to-sp=false"
        ]
    ):
        res = run_bass_kernel_spmd(
            nc, in_maps, core_ids=list(range(8)), trace=trace, **trace_kwargs
        )
    bp = np.asarray(bp, dtype=np.float32)
    y = np.empty((2, T, EMB), dtype=np.float32)
    for b in range(2):
        acc = res.results[4 * b]["partial"].astype(np.float32)
        for g in range(1, 4):
            acc += res.results[4 * b + g]["partial"].astype(np.float32)
        y[b] = acc + bp
    return y, res


def kernel(x, Wq, Wk, Wv, Wp, bp):
    y, _ = run(x, Wq, Wk, Wv, Wp, bp, trace=False)
    return y
